# revision 1
# baseline (speedup 1.0000x reference)
"""Trainium2 Bass kernel for nn_BilinearSparseRouting (FC capsule routing layer).

Math (after constant-folding the softmax-over-a-constant, which is exactly 1/32):
    cp2[b,j]   = (pose[b,j] as 4x4) @ wc[j]            # (4,4) each
    S[b]       = (1/32) * sum_j cp2[b,j]               # (4,4)
    out[b,o]   = S[b] @ wn[o]                          # (4,4), o = 0..31
    output shape (256, 1, 1, 32, 16)

Device strategy (data-parallel over batch, 32 batches per core):
  Stage 1 is a 16384-term contraction per (b, r):
      T[(b,r), c] = sum_{(j,k)} pose[b, j, 4r+k] * wc[j, k, c]

  fp32 matmuls on the PE run as 2 half-speed passes (4 cyc/col); instead we
  split both operands into fp16 hi+lo pairs (x = x_hi + x_lo captures 22
  mantissa bits; fp16*fp16 products are exact in the fp32 PSUM accumulate).
  Packing rhs = [x_hi | x_lo] (256 cols) and lhsT = [w_hi | w_lo] (8 cols)
  computes all four cross products in ONE fp16 matmul per 128-row chunk:

      psum1[c + 4*hw, (b,r) + 128*hx] += w_hw_chunk.T @ x_hx_chunk

  at 1 cyc/col -- 4x faster than fp32 with ~fp32 accuracy (the lo*lo term
  is included, cutting the residual to ~2^-22).

  Stage 2 folds the quadrant collapse + the exact 1/32 scale into a single
  contraction against wn/32 replicated over the hw axis:

      out[(b,r),(o,c)] = sum_{k2,hw} s8[(k2,hw), (b,r)+128*hx] * wn8[(k2,hw),(o,c)]

  accumulated over hx in 2 tiny fp32 matmuls.

  The 8 MiB/core x stream is laid out on the host as per-group dense
  contiguous DRAM regions and streamed on the sync HWDGE ring with all
  destination tiles SBUF-resident, so the DMAs queue back-to-back at the
  practical HBM rate (~295 GB/s/core measured with all 8 cores streaming).
"""

import os
import sys

for _p in ("/opt/trn_rl_repo", "/root/.axon_site/_ro/trn_rl_repo"):
    if _p not in sys.path:
        sys.path.insert(0, _p)

# The kernel executes through the axon PJRT backend; a leftover cpu pin from a
# reference-running harness would hide the NeuronCores if jax has not
# initialized its backend yet.
os.environ.pop("JAX_PLATFORMS", None)

from contextlib import ExitStack  # noqa: E402

import numpy as np  # noqa: E402

import concourse.bacc as bacc  # noqa: E402
import concourse.mybir as mybir  # noqa: E402
import concourse.tile as tile  # noqa: E402
from concourse.bass_utils import run_bass_kernel_spmd  # noqa: E402

B = 256
N_IN = 4096
N_OUT = 32
MPD = 4
POSE_DIM = 16
N_CORES = 8
B_SH = B // N_CORES            # 32 batches per core
JK = N_IN * MPD                # 16384 contraction terms
NCHUNK = JK // 128             # 128 PE matmuls
XCOLS = NCHUNK * 256           # fp16 hi|lo packed columns of x

F32 = mybir.dt.float32
F16 = mybir.dt.float16

# Built once, reused across kernel() calls.
_CACHE = {}

# test.py hooks: set TRACE=True before calling kernel() to profile; the
# BassKernelResults of the last run lands in LAST_RESULT.
TRACE = False
TRACE_KWARGS = {}
LAST_RESULT = None


def _build_program():
    nc = bacc.Bacc("TRN2", target_bir_lowering=False, debug=False,
                   num_devices=N_CORES)
    wn = nc.dram_tensor("wn", [8, N_OUT * MPD], F32, kind="ExternalInput").ap()
    y = nc.dram_tensor("y", [128, 128], F32, kind="ExternalOutput").ap()

    # Group boundaries in chunks: small first group so the matmul stream
    # starts early, then a geometrically tapering tail.  A group's matmuls
    # can only start once its whole DMA lands (sem granularity), and the PE
    # consumes ~0.11 us/chunk vs ~0.18 us/chunk delivery -- so each trailing
    # group at <= ~1.4x the size of the next keeps the PE finishing a group
    # right as the next lands, cutting the post-stream PE trail from ~3 us
    # (one 30-chunk group) to ~0.5 us.
    bounds = [0, 4, 32, 60, 84, 100, 112, 119, 123, 126, 128]
    assert bounds[-1] == NCHUNK

    # One DRAM tensor per stream group: each group is a dense contiguous
    # region (partition stride = the group's row length), giving the HBM
    # reads a compact footprint instead of 64 KiB-strided rows.  Group 0
    # carries the stage-1 weights prepended to its columns, so one DMA
    # delivers everything the first matmuls need.
    W8 = NCHUNK * 8
    xg = [
        nc.dram_tensor(
            f"x{g}",
            [128, (bounds[g + 1] - bounds[g]) * 256 + (W8 if g == 0 else 0)],
            F16, kind="ExternalInput").ap()
        for g in range(len(bounds) - 1)
    ]

    with tile.TileContext(nc) as tc, ExitStack() as ctx:
        wpool = ctx.enter_context(tc.tile_pool(name="wpool", bufs=1))
        # All x groups stay resident (8 MiB) so every stream DMA can be
        # issued up front; the sync HWDGE ring then drains back-to-back at
        # the HBM rate with no buffer-release gating.
        xpool = ctx.enter_context(tc.tile_pool(name="xpool", bufs=1))
        opool = ctx.enter_context(tc.tile_pool(name="opool", bufs=1))
        ppool = ctx.enter_context(tc.tile_pool(name="ppool", bufs=1, space="PSUM"))

        wn_sb = wpool.tile([8, N_OUT * MPD], F32, tag="wn")
        nc.scalar.dma_start(wn_sb[:], wn[:])

        # Stage 1: one fp16 matmul per 128-row chunk covers all 4 hi/lo
        # cross products; accumulate everything into one (8, 256) psum.
        psum1 = ppool.tile([8, 256], F32, tag="t")
        xts = []
        n_groups = len(bounds) - 1
        for g in range(n_groups):
            c0, c1 = bounds[g], bounds[g + 1]
            ncols = (c1 - c0) * 256 + (W8 if g == 0 else 0)
            xt = xpool.tile([128, ncols], F16, tag=f"x{g}")
            nc.sync.dma_start(xt[:], xg[g][:])
            xts.append(xt)
        w_sb = xts[0][:, 0:W8]
        for g in range(n_groups):
            c0, c1 = bounds[g], bounds[g + 1]
            xt = xts[g]
            off = W8 if g == 0 else 0
            for jj in range(c1 - c0):
                c = c0 + jj
                nc.tensor.matmul(
                    psum1[:],
                    lhsT=w_sb[:, c * 8:(c + 1) * 8],
                    rhs=xt[:, off + jj * 256:off + (jj + 1) * 256],
                    start=(c == 0),
                    stop=(c == NCHUNK - 1),
                )

        s8 = opool.tile([8, 256], F32, tag="s8")
        nc.vector.tensor_copy(s8[:], psum1[:])

        # Stage 2: contract over (k2, hw) against wn/32 (host-prescaled,
        # exact power-of-2), accumulating the two hx halves.
        psum2 = ppool.tile([128, 128], F32, tag="out")
        nc.tensor.matmul(psum2[:], lhsT=s8[:, 0:128], rhs=wn_sb[:],
                         start=True, stop=False)
        nc.tensor.matmul(psum2[:], lhsT=s8[:, 128:256], rhs=wn_sb[:],
                         start=False, stop=True)
        out_sb = opool.tile([128, 128], F32, tag="y")
        nc.vector.tensor_copy(out_sb[:], psum2[:])
        nc.sync.dma_start(y[:], out_sb[:])

    nc.compile()
    _CACHE["bounds"] = bounds
    return nc


def _split_f16(a: np.ndarray):
    hi = a.astype(np.float16)
    lo = (a - hi.astype(np.float32)).astype(np.float16)
    return hi, lo


def _prep_x(current_pose: np.ndarray) -> np.ndarray:
    """(256, 4096, 16) -> (8 cores, 128 partitions, NCHUNK*256 fp16 cols).

    Per core the stage-1 contraction matrix has row index (j*4 + k) and
    column (b*4 + r) with element pose[b, j, 4r+k].  Chunk Jc's 128x128
    tile lands in packed columns [Jc*256, Jc*256+128) as fp16 hi and
    [Jc*256+128, (Jc+1)*256) as fp16 lo.
    """
    a = current_pose.reshape(N_CORES, B_SH, N_IN, MPD, MPD)   # m b j r k
    t = a.transpose(0, 2, 4, 1, 3)                            # m j k b r
    c = t.reshape(N_CORES, NCHUNK, 128, 128)                  # m Jc p col
    c = np.ascontiguousarray(c.transpose(0, 2, 1, 3))         # m p Jc col
    hi, lo = _split_f16(c)
    packed = np.stack([hi, lo], axis=3)                       # m p Jc {hi,lo} col
    return np.ascontiguousarray(packed.reshape(N_CORES, 128, XCOLS))


def kernel(current_pose, w_current, w_next, h_out=1, w_out=1):
    global LAST_RESULT
    current_pose = np.asarray(current_pose, dtype=np.float32)
    w_current = np.asarray(w_current, dtype=np.float32)
    w_next = np.asarray(w_next, dtype=np.float32)

    if not TRACE:
        # bass_utils would honor a stray BASS_TRACE env var and then crash on
        # this image's missing NTFF hook module.
        os.environ.pop("BASS_TRACE", None)

    if "nc" not in _CACHE:
        _CACHE["nc"] = _build_program()
    nc = _CACHE["nc"]
    bounds = _CACHE["bounds"]

    xs = _prep_x(current_pose)

    # wc[j,k,c] flattened over rows (j,k); chunk Jc's (128, 4) block packed
    # into SBUF-image columns [Jc*8, Jc*8+4) as fp16 hi, [Jc*8+4, +8) as lo.
    wc_flat = w_current.reshape(JK, MPD)
    whi, wlo = _split_f16(wc_flat)
    w_img = np.concatenate(
        [whi.reshape(NCHUNK, 128, MPD), wlo.reshape(NCHUNK, 128, MPD)], axis=2)
    w_img = np.ascontiguousarray(
        w_img.transpose(1, 0, 2).reshape(128, NCHUNK * 8))

    # wn arranged (k2, (o,c)), pre-scaled by the exact 1/32 softmax constant
    # and replicated over the w-hi/lo axis for the stage-2 collapse.
    wn_t = (w_next.transpose(1, 0, 2).reshape(MPD, N_OUT * MPD)
            * np.float32(1.0 / N_OUT))
    wn8 = np.ascontiguousarray(np.concatenate([wn_t, wn_t], axis=0))

    in_maps = [
        {"wn": wn8,
         "x0": np.ascontiguousarray(np.concatenate(
             [w_img, xs[m][:, bounds[0] * 256:bounds[1] * 256]], axis=1)),
         **{f"x{g}": np.ascontiguousarray(
                xs[m][:, bounds[g] * 256:bounds[g + 1] * 256])
            for g in range(1, len(bounds) - 1)}}
        for m in range(N_CORES)
    ]
    res = run_bass_kernel_spmd(nc, in_maps, list(range(N_CORES)), trace=TRACE,
                               **TRACE_KWARGS)
    LAST_RESULT = res

    out = np.empty((B, 1, 1, N_OUT, POSE_DIM), dtype=np.float32)
    for m in range(N_CORES):
        ym = res.results[m]["y"]                      # (128=(b,r), 128=(o,c))
        out[m * B_SH:(m + 1) * B_SH, 0, 0] = (
            ym.reshape(B_SH, MPD, N_OUT, MPD)
            .transpose(0, 2, 1, 3).reshape(B_SH, N_OUT, POSE_DIM))
    return out



# revision 6
# speedup vs baseline: 1.1710x; 1.1710x over previous
"""Trainium2 Bass kernel for nn_BilinearSparseRouting (FC capsule routing layer).

Math (after constant-folding the softmax-over-a-constant, which is exactly 1/32):
    cp2[b,j]   = (pose[b,j] as 4x4) @ wc[j]            # (4,4) each
    S[b]       = (1/32) * sum_j cp2[b,j]               # (4,4)
    out[b,o]   = S[b] @ wn[o]                          # (4,4), o = 0..31
    output shape (256, 1, 1, 32, 16)

Device strategy (data-parallel over batch, 32 batches per core):
  Stage 1 is a 16384-term contraction per (b, r):
      T[(b,r), c] = sum_{(j,k)} pose[b, j, 4r+k] * wc[j, k, c]

  The end-to-end tolerance (2e-2) dwarfs fp16 rounding (~3e-4 through this
  contraction), so both operands stream as SINGLE fp16 values -- half the
  HBM bytes of an fp32 or hi/lo-pair scheme.  The kernel is then HBM-bound:
  ~4.3 MiB/core of pose data at the ~380 B/ns effective per-core rate.

  PE structure: chunks of 128 contraction rows are PAIRED into one matmul,
      psum1[8, 256] += [wc_2p | wc_2p+1].T @ [x_2p | x_2p+1]
  so only the diagonal quadrants (0:4, 0:128) and (4:8, 128:256) carry the
  even/odd partial sums (off-diagonal quadrants are garbage and never
  read).  This keeps the baseline-proven 256-column instruction cadence
  (~213 ns sustained) with 64 matmuls instead of 128, and the fp16 columns
  stream at 1 col/cycle -- PE ingest roughly matches HBM delivery.

  Stage 2 compacts the two live quadrants into a [4, 256] fp16 tile and
  contracts against wn/32 (host-prescaled, exact power of 2) in two small
  fp16 matmuls accumulating into one [128, 128] psum.

  The x stream is laid out on the host as per-group dense contiguous DRAM
  regions; group doorbells alternate between the sync and scalar HWDGE
  rings so descriptor issue (~0.6 us per DMA_DIRECT2D) is two-wide and the
  stream saturates the DMA engines with all destination tiles
  SBUF-resident.  Group 0 carries the stage-1 weights and wn so one early
  DMA delivers everything the first matmuls and stage 2 need.
"""

import os
import sys

for _p in ("/opt/trn_rl_repo", "/root/.axon_site/_ro/trn_rl_repo"):
    if _p not in sys.path:
        sys.path.insert(0, _p)

# The kernel executes through the axon PJRT backend; a leftover cpu pin from a
# reference-running harness would hide the NeuronCores if jax has not
# initialized its backend yet.
os.environ.pop("JAX_PLATFORMS", None)

from contextlib import ExitStack  # noqa: E402

import numpy as np  # noqa: E402

import concourse.bacc as bacc  # noqa: E402
import concourse.mybir as mybir  # noqa: E402
import concourse.tile as tile  # noqa: E402
from concourse.bass_utils import run_bass_kernel_spmd  # noqa: E402

B = 256
N_IN = 4096
N_OUT = 32
MPD = 4
POSE_DIM = 16
N_CORES = 8
B_SH = B // N_CORES            # 32 batches per core
JK = N_IN * MPD                # 16384 contraction terms
NCHUNK = JK // 128             # 128 contraction chunks of 128 rows
NPAIR = NCHUNK // 2            # 64 pair matmuls
XCOLS = NCHUNK * 128           # fp16 packed columns of x
W4 = NCHUNK * 4                # stage-1 weight columns (4 per chunk)
WNC = 256                      # wn block columns in group 0 (2 parity blocks)

F32 = mybir.dt.float32
F16 = mybir.dt.float16

# Built once, reused across kernel() calls.
_CACHE = {}

# test.py hooks: set TRACE=True before calling kernel() to profile; the
# BassKernelResults of the last run lands in LAST_RESULT.
TRACE = False
TRACE_KWARGS = {}
LAST_RESULT = None

# Group boundaries in chunks (all deltas even so pair matmuls never span a
# group).  Small first group so the matmul stream starts early; mild taper
# at the tail so the last chunks land just before the PE needs them.
BOUNDS = [0, 2, 10, 24, 40, 56, 72, 88, 104, 118, 128]


def _build_program():
    nc = bacc.Bacc("TRN2", target_bir_lowering=False, debug=False,
                   num_devices=N_CORES)
    y = nc.dram_tensor("y", [128, 128], F32, kind="ExternalOutput").ap()

    bounds = BOUNDS
    assert bounds[-1] == NCHUNK

    # One DRAM tensor per stream group: each group is a dense contiguous
    # region (partition stride = the group's row length), giving the HBM
    # reads a compact footprint.  Group 0 carries the stage-1 weights and
    # the wn block prepended to its columns.
    hdr = W4 + WNC
    xg = [
        nc.dram_tensor(
            f"x{g}",
            [128, (bounds[g + 1] - bounds[g]) * 128 + (hdr if g == 0 else 0)],
            F16, kind="ExternalInput").ap()
        for g in range(len(bounds) - 1)
    ]

    with tile.TileContext(nc) as tc, ExitStack() as ctx:
        # All x groups stay resident (4.3 MiB) so every stream DMA can be
        # issued up front; the two HWDGE rings then drain back-to-back at
        # the HBM rate with no buffer-release gating.
        xpool = ctx.enter_context(tc.tile_pool(name="xpool", bufs=1))
        opool = ctx.enter_context(tc.tile_pool(name="opool", bufs=1))
        ppool = ctx.enter_context(tc.tile_pool(name="ppool", bufs=1, space="PSUM"))

        n_groups = len(bounds) - 1
        xts = []
        for g in range(n_groups):
            c0, c1 = bounds[g], bounds[g + 1]
            ncols = (c1 - c0) * 128 + (hdr if g == 0 else 0)
            xt = xpool.tile([128, ncols], F16, tag=f"x{g}")
            # Alternate doorbell issue between the sync and scalar HWDGE
            # rings so descriptor writes (~0.6 us each) overlap.
            eng = nc.sync if g % 2 == 0 else nc.scalar
            eng.dma_start(xt[:], xg[g][:])
            xts.append(xt)
        w_sb = xts[0][:, 0:W4]
        wn_sb = xts[0][0:8, W4:W4 + WNC]

        # Stage 1: 64 paired 256-column fp16 matmuls accumulate the even
        # chunks' partial S into psum quadrant (0:4, 0:128) and the odd
        # chunks' into (4:8, 128:256).
        psum1 = ppool.tile([8, 256], F32, tag="t")
        for g in range(n_groups):
            c0, c1 = bounds[g], bounds[g + 1]
            xt = xts[g]
            off = hdr if g == 0 else 0
            for pp in range((c1 - c0) // 2):
                p = c0 // 2 + pp
                nc.tensor.matmul(
                    psum1[:],
                    lhsT=w_sb[:, p * 8:(p + 1) * 8],
                    rhs=xt[:, off + pp * 256:off + (pp + 1) * 256],
                    start=(p == 0),
                    stop=(p == NPAIR - 1),
                )

        # Downcast the full psum (live + garbage quadrants) to fp16 in one
        # partition-0-aligned copy; compute engines cannot start an access
        # at partition 4, so the garbage quadrants are neutralized in stage
        # 2 instead, by zero rows in the wn operand.
        s8 = opool.tile([8, 256], F16, tag="s8")
        nc.vector.tensor_copy(s8[:], psum1[:])

        # Stage 2: contract over (k2, parity) against wn/32.  The wn block
        # for the even half has wn in rows 0:4 and zeros in rows 4:8 (and
        # vice versa for the odd half), so the garbage quadrants of s8
        # multiply exact zeros.
        psum2 = ppool.tile([128, 128], F32, tag="out")
        nc.tensor.matmul(psum2[:], lhsT=s8[:, 0:128], rhs=wn_sb[:, 0:128],
                         start=True, stop=False)
        nc.tensor.matmul(psum2[:], lhsT=s8[:, 128:256], rhs=wn_sb[:, 128:256],
                         start=False, stop=True)
        out_sb = opool.tile([128, 128], F32, tag="y")
        nc.vector.tensor_copy(out_sb[:], psum2[:])
        nc.sync.dma_start(y[:], out_sb[:])

    nc.compile()
    return nc


def _prep_x(current_pose: np.ndarray) -> np.ndarray:
    """(256, 4096, 16) -> (8 cores, 128 partitions, NCHUNK*128 fp16 cols).

    Per core the stage-1 contraction matrix has row index (j*4 + k) and
    column (b*4 + r) with element pose[b, j, 4r+k].  Chunk Jc's 128x128
    tile lands in packed columns [Jc*128, (Jc+1)*128).
    """
    a = current_pose.reshape(N_CORES, B_SH, N_IN, MPD, MPD)   # m b j r k
    t = a.transpose(0, 2, 4, 1, 3)                            # m j k b r
    c = t.reshape(N_CORES, NCHUNK, 128, 128)                  # m Jc p col
    c = c.transpose(0, 2, 1, 3)                               # m p Jc col
    return np.ascontiguousarray(
        c.reshape(N_CORES, 128, XCOLS).astype(np.float16))


def kernel(current_pose, w_current, w_next, h_out=1, w_out=1):
    global LAST_RESULT
    current_pose = np.asarray(current_pose, dtype=np.float32)
    w_current = np.asarray(w_current, dtype=np.float32)
    w_next = np.asarray(w_next, dtype=np.float32)

    if not TRACE:
        # bass_utils would honor a stray BASS_TRACE env var and then crash on
        # this image's missing NTFF hook module.
        os.environ.pop("BASS_TRACE", None)

    if "nc" not in _CACHE:
        _CACHE["nc"] = _build_program()
    nc = _CACHE["nc"]
    bounds = BOUNDS

    xs = _prep_x(current_pose)

    # wc[j,k,c] flattened over rows (j,k); chunk Jc's (128, 4) block packed
    # into SBUF-image columns [Jc*4, (Jc+1)*4).
    wc_flat = w_current.reshape(JK, MPD).astype(np.float16)
    w_img = np.ascontiguousarray(
        wc_flat.reshape(NCHUNK, 128, MPD).transpose(1, 0, 2).reshape(128, W4))

    # wn arranged (k2, (o,c)), pre-scaled by the exact 1/32 softmax
    # constant, in two parity blocks: even block rows 0:4, odd block rows
    # 4:8; the complementary rows stay zero to kill the psum1 garbage
    # quadrants in stage 2.
    wn4 = (w_next.transpose(1, 0, 2).reshape(MPD, N_OUT * MPD)
           * np.float32(1.0 / N_OUT)).astype(np.float16)
    wn_img = np.zeros((128, WNC), dtype=np.float16)
    wn_img[0:MPD, 0:128] = wn4
    wn_img[MPD:2 * MPD, 128:256] = wn4

    in_maps = [
        {"x0": np.ascontiguousarray(np.concatenate(
             [w_img, wn_img, xs[m][:, bounds[0] * 128:bounds[1] * 128]],
             axis=1)),
         **{f"x{g}": np.ascontiguousarray(
                xs[m][:, bounds[g] * 128:bounds[g + 1] * 128])
            for g in range(1, len(bounds) - 1)}}
        for m in range(N_CORES)
    ]
    res = run_bass_kernel_spmd(nc, in_maps, list(range(N_CORES)), trace=TRACE,
                               **TRACE_KWARGS)
    LAST_RESULT = res

    out = np.empty((B, 1, 1, N_OUT, POSE_DIM), dtype=np.float32)
    for m in range(N_CORES):
        ym = res.results[m]["y"]                      # (128=(b,r), 128=(o,c))
        out[m * B_SH:(m + 1) * B_SH, 0, 0] = (
            ym.reshape(B_SH, MPD, N_OUT, MPD)
            .transpose(0, 2, 1, 3).reshape(B_SH, N_OUT, POSE_DIM))
    return out


# revision 11
# speedup vs baseline: 1.3165x; 1.1242x over previous
"""Trainium2 Bass kernel for nn_BilinearSparseRouting (FC capsule routing layer).

Math (after constant-folding the softmax-over-a-constant, which is exactly 1/32):
    cp2[b,j]   = (pose[b,j] as 4x4) @ wc[j]            # (4,4) each
    S[b]       = (1/32) * sum_j cp2[b,j]               # (4,4)
    out[b,o]   = S[b] @ wn[o]                          # (4,4), o = 0..31
    output shape (256, 1, 1, 32, 16)

Device strategy (data-parallel over batch, 32 batches per core):
  Stage 1 is a 16384-term contraction per (b, r):
      T[(b,r), c] = sum_{(j,k)} pose[b, j, 4r+k] * wc[j, k, c]

  The end-to-end tolerance (2e-2) dwarfs fp16 rounding (~3e-4 through this
  contraction), so both operands stream as SINGLE fp16 values -- half the
  HBM bytes of an fp32 or hi/lo-pair scheme.  The kernel is then HBM-bound:
  ~4.3 MiB/core of pose data at the ~380 B/ns effective per-core rate.

  PE structure: chunks of 128 contraction rows are PAIRED into one matmul,
      psum1[8, 256] += [wc_2p | wc_2p+1].T @ [x_2p | x_2p+1]
  so only the diagonal quadrants (0:4, 0:128) and (4:8, 128:256) carry the
  even/odd partial sums (off-diagonal quadrants are garbage and never
  read).  This keeps the baseline-proven 256-column instruction cadence
  (~213 ns sustained) with 64 matmuls instead of 128, and the fp16 columns
  stream at 1 col/cycle -- PE ingest roughly matches HBM delivery.

  Stage 2 compacts the two live quadrants into a [4, 256] fp16 tile and
  contracts against wn/32 (host-prescaled, exact power of 2) in two small
  fp16 matmuls accumulating into one [128, 128] psum.

  The x stream is laid out on the host as per-group dense contiguous DRAM
  regions; group doorbells alternate between the sync and scalar HWDGE
  rings so descriptor issue (~0.6 us per DMA_DIRECT2D) is two-wide and the
  stream saturates the DMA engines with all destination tiles
  SBUF-resident.  Group 0 carries the stage-1 weights and wn so one early
  DMA delivers everything the first matmuls and stage 2 need.
"""

import os
import sys

for _p in ("/opt/trn_rl_repo", "/root/.axon_site/_ro/trn_rl_repo"):
    if _p not in sys.path:
        sys.path.insert(0, _p)

# The kernel executes through the axon PJRT backend; a leftover cpu pin from a
# reference-running harness would hide the NeuronCores if jax has not
# initialized its backend yet.
os.environ.pop("JAX_PLATFORMS", None)

from contextlib import ExitStack  # noqa: E402

import numpy as np  # noqa: E402

import concourse.bacc as bacc  # noqa: E402
import concourse.mybir as mybir  # noqa: E402
import concourse.tile as tile  # noqa: E402
from concourse.bass_utils import run_bass_kernel_spmd  # noqa: E402

B = 256
N_IN = 4096
N_OUT = 32
MPD = 4
POSE_DIM = 16
N_CORES = 8
B_SH = B // N_CORES            # 32 batches per core
JK = N_IN * MPD                # 16384 contraction terms
NCHUNK = JK // 128             # 128 contraction chunks of 128 rows
NPAIR = NCHUNK // 2            # 64 pair matmuls
XCOLS = NCHUNK * 128           # fp16 packed columns of x
W4 = NCHUNK * 4                # stage-1 weight columns (4 per chunk)
WNC = 256                      # wn block columns in group 0 (2 parity blocks)

F32 = mybir.dt.float32
F16 = mybir.dt.float16

# Built once, reused across kernel() calls.
_CACHE = {}

# test.py hooks: set TRACE=True before calling kernel() to profile; the
# BassKernelResults of the last run lands in LAST_RESULT.
TRACE = False
TRACE_KWARGS = {}
LAST_RESULT = None

# Group boundaries in chunks (all deltas even so pair matmuls never span a
# group).  Group 0 is the weight header alone so the first x chunks follow
# immediately behind it on the same ring; delivery is in consumption order.
BOUNDS = [0, 0, 8, 22, 38, 54, 70, 86, 102, 116, 128]

# Dummy 256-column matmuls on uninitialized SBUF, run while the stream's
# first groups are still in flight: the PE p-state ramps with busy time
# (1.2 -> 2.4 GHz), and a cold PE runs stage 1 at half the column rate for
# several microseconds.  Results land in a scratch psum nobody reads.
N_WARM = 8


def _build_program():
    nc = bacc.Bacc("TRN2", target_bir_lowering=False, debug=False,
                   num_devices=N_CORES)
    y = nc.dram_tensor("y", [128, 128], F32, kind="ExternalOutput").ap()

    bounds = BOUNDS
    assert bounds[-1] == NCHUNK

    # One DRAM tensor per stream group: each group is a dense contiguous
    # region (partition stride = the group's row length), giving the HBM
    # reads a compact footprint.  Group 0 carries the stage-1 weights and
    # the wn block prepended to its columns.
    hdr = W4 + WNC
    xg = [
        nc.dram_tensor(
            f"x{g}",
            [128, (bounds[g + 1] - bounds[g]) * 128 + (hdr if g == 0 else 0)],
            F16, kind="ExternalInput").ap()
        for g in range(len(bounds) - 1)
    ]

    with tile.TileContext(nc) as tc, ExitStack() as ctx:
        # All x groups stay resident (4.3 MiB) so every stream DMA can be
        # issued up front; the two HWDGE rings then drain back-to-back at
        # the HBM rate with no buffer-release gating.
        xpool = ctx.enter_context(tc.tile_pool(name="xpool", bufs=1))
        opool = ctx.enter_context(tc.tile_pool(name="opool", bufs=1))
        ppool = ctx.enter_context(tc.tile_pool(name="ppool", bufs=1, space="PSUM"))

        n_groups = len(bounds) - 1
        xts = []
        for g in range(n_groups):
            c0, c1 = bounds[g], bounds[g + 1]
            ncols = (c1 - c0) * 128 + (hdr if g == 0 else 0)
            xt = xpool.tile([128, ncols], F16, tag=f"x{g}")
            # All groups ride the sync HWDGE ring: a second ring does not
            # add bandwidth (the 16 DMA engines are shared) and delivering
            # groups out of consumption order stalls the PE.
            nc.sync.dma_start(xt[:], xg[g][:])
            xts.append(xt)
        w_sb = xts[0][:, 0:W4]
        wn_sb = xts[0][0:8, W4:W4 + WNC]

        # PE warm-up: depends only on a scalar-engine memzero, so these
        # issue right after the engine prologues, several us before the
        # first x group lands.  The zero products stay in a scratch psum
        # that is never read.
        warm = opool.tile([128, 256], F16, tag="warm")
        nc.scalar.memzero(warm[:])
        psum_w = ppool.tile([8, 256], F32, tag="warmp")
        for i in range(N_WARM):
            nc.tensor.matmul(psum_w[:], lhsT=warm[:, 0:8], rhs=warm[:],
                             start=(i == 0), stop=(i == N_WARM - 1))

        # Stage 1: 64 paired 256-column fp16 matmuls accumulate the even
        # chunks' partial S into psum quadrant (0:4, 0:128) and the odd
        # chunks' into (4:8, 128:256).
        psum1 = ppool.tile([8, 256], F32, tag="t")
        for g in range(n_groups):
            c0, c1 = bounds[g], bounds[g + 1]
            xt = xts[g]
            off = hdr if g == 0 else 0
            for pp in range((c1 - c0) // 2):
                p = c0 // 2 + pp
                nc.tensor.matmul(
                    psum1[:],
                    lhsT=w_sb[:, p * 8:(p + 1) * 8],
                    rhs=xt[:, off + pp * 256:off + (pp + 1) * 256],
                    start=(p == 0),
                    stop=(p == NPAIR - 1),
                )

        # Downcast the full psum (live + garbage quadrants) to fp16 in one
        # partition-0-aligned copy; compute engines cannot start an access
        # at partition 4, so the garbage quadrants are neutralized in stage
        # 2 instead, by zero rows in the wn operand.
        s8 = opool.tile([8, 256], F16, tag="s8")
        nc.vector.tensor_copy(s8[:], psum1[:])

        # Stage 2: contract over (k2, parity) against wn/32.  The wn block
        # for the even half has wn in rows 0:4 and zeros in rows 4:8 (and
        # vice versa for the odd half), so the garbage quadrants of s8
        # multiply exact zeros.
        psum2 = ppool.tile([128, 128], F32, tag="out")
        nc.tensor.matmul(psum2[:], lhsT=s8[:, 0:128], rhs=wn_sb[:, 0:128],
                         start=True, stop=False)
        nc.tensor.matmul(psum2[:], lhsT=s8[:, 128:256], rhs=wn_sb[:, 128:256],
                         start=False, stop=True)
        out_sb = opool.tile([128, 128], F32, tag="y")
        nc.vector.tensor_copy(out_sb[:], psum2[:])
        nc.sync.dma_start(y[:], out_sb[:])

    nc.compile()
    return nc


def _prep_x(current_pose: np.ndarray) -> np.ndarray:
    """(256, 4096, 16) -> (8 cores, 128 partitions, NCHUNK*128 fp16 cols).

    Per core the stage-1 contraction matrix has row index (j*4 + k) and
    column (b*4 + r) with element pose[b, j, 4r+k].  Chunk Jc's 128x128
    tile lands in packed columns [Jc*128, (Jc+1)*128).
    """
    a = current_pose.reshape(N_CORES, B_SH, N_IN, MPD, MPD)   # m b j r k
    t = a.transpose(0, 2, 4, 1, 3)                            # m j k b r
    c = t.reshape(N_CORES, NCHUNK, 128, 128)                  # m Jc p col
    c = c.transpose(0, 2, 1, 3)                               # m p Jc col
    return np.ascontiguousarray(
        c.reshape(N_CORES, 128, XCOLS).astype(np.float16))


def kernel(current_pose, w_current, w_next, h_out=1, w_out=1):
    global LAST_RESULT
    current_pose = np.asarray(current_pose, dtype=np.float32)
    w_current = np.asarray(w_current, dtype=np.float32)
    w_next = np.asarray(w_next, dtype=np.float32)

    if not TRACE:
        # bass_utils would honor a stray BASS_TRACE env var and then crash on
        # this image's missing NTFF hook module.
        os.environ.pop("BASS_TRACE", None)

    if "nc" not in _CACHE:
        _CACHE["nc"] = _build_program()
    nc = _CACHE["nc"]
    bounds = BOUNDS

    xs = _prep_x(current_pose)

    # wc[j,k,c] flattened over rows (j,k); chunk Jc's (128, 4) block packed
    # into SBUF-image columns [Jc*4, (Jc+1)*4).
    wc_flat = w_current.reshape(JK, MPD).astype(np.float16)
    w_img = np.ascontiguousarray(
        wc_flat.reshape(NCHUNK, 128, MPD).transpose(1, 0, 2).reshape(128, W4))

    # wn arranged (k2, (o,c)), pre-scaled by the exact 1/32 softmax
    # constant, in two parity blocks: even block rows 0:4, odd block rows
    # 4:8; the complementary rows stay zero to kill the psum1 garbage
    # quadrants in stage 2.
    wn4 = (w_next.transpose(1, 0, 2).reshape(MPD, N_OUT * MPD)
           * np.float32(1.0 / N_OUT)).astype(np.float16)
    wn_img = np.zeros((128, WNC), dtype=np.float16)
    wn_img[0:MPD, 0:128] = wn4
    wn_img[MPD:2 * MPD, 128:256] = wn4

    in_maps = [
        {"x0": np.ascontiguousarray(np.concatenate(
             [w_img, wn_img, xs[m][:, bounds[0] * 128:bounds[1] * 128]],
             axis=1)),
         **{f"x{g}": np.ascontiguousarray(
                xs[m][:, bounds[g] * 128:bounds[g + 1] * 128])
            for g in range(1, len(bounds) - 1)}}
        for m in range(N_CORES)
    ]
    res = run_bass_kernel_spmd(nc, in_maps, list(range(N_CORES)), trace=TRACE,
                               **TRACE_KWARGS)
    LAST_RESULT = res

    out = np.empty((B, 1, 1, N_OUT, POSE_DIM), dtype=np.float32)
    for m in range(N_CORES):
        ym = res.results[m]["y"]                      # (128=(b,r), 128=(o,c))
        out[m * B_SH:(m + 1) * B_SH, 0, 0] = (
            ym.reshape(B_SH, MPD, N_OUT, MPD)
            .transpose(0, 2, 1, 3).reshape(B_SH, N_OUT, POSE_DIM))
    return out
